# revision 10
# baseline (speedup 1.0000x reference)
"""GPT-NeoX attention layer as a Bass/Tile kernel for 8 Trainium2 NeuronCores.

Problem: hidden[2048,1,4096] -> QKV proj (W[4096,12288]) -> 32-head attention
(head_dim 128, rotary on first 32 dims, causal) -> dense proj (W[4096,4096]).

Sharding: tensor-parallel over heads (4 heads/core). Each core:
  P0: PE-transposes its 512-column shard of hidden; quantizes to an fp8e4
      pair (x8 + 64*residual) and AllGathers both -> full hidden^T.
  P1: QKV projection with fp8 DoubleRow matmuls (contraction 256/instr).
      Each GEMM runs 3 error-compensated passes:
        hi = W8.x8, lo = W8.xr8 + Wr8.x8, result = (hi + lo/64)/64
      where W8 = fp8(64 W), Wr8 = fp8(64(64W - W8)), x8 = fp8(x),
      xr8 = fp8(64(x - x8)). q/k produced TRANSPOSED ([head_dim, seq]) via a
      host-side column permutation; rotary applied on-chip (rotate_half via a
      partition-permuting SBUF->SBUF DMA, sign baked into the sin table);
      v produced in [seq, head_dim] fp16.
  P2: attention per head: scores^T tiles [kv 128 x q 512] on PE (fp16 q/k),
      additive causal mask, exp on ScalarE, denominator via ones-matmul,
      PV matmul (fp16 v stationary, f32r probs moving) accumulates ctx^T;
      ctx normalized then quantized to the fp8 pair (ctx8, 64*resid).
  P3: AllGather ctx pair -> full [2*4096, 2048] (per i-block).
  P4: dense projection, column-sharded, fp8 DoubleRow 3-pass.
Host gathers by concatenating the 8 column slices.
"""
import sys
import os

sys.path.insert(0, "/opt/trn_rl_repo")

import numpy as np

import concourse.bacc as bacc
import concourse.mybir as mybir
import concourse.tile as tile

SEQ = 2048
HIDDEN = 4096
HEADS = 32
HD = 128
ROT = 32
HALF = ROT // 2  # 16
N_CORES = 8
HPC = HEADS // N_CORES       # 4 heads per core
CW = HPC * HD                # 512 columns of work per core (v / ctx / dense)
KT = HIDDEN // 128           # 32 k-tiles over the hidden dim
NS = KT // 2                 # 16 DoubleRow k-slabs (256-deep)
SB = 512                     # sequence block for QKV + attention i-blocks
NSB = SEQ // SB              # 4
NST = SEQ // 128             # 16 sequence tiles
NEG = -1.0e9                 # additive mask value (pre-scale)
SCALE = float(1.0 / np.sqrt(HD))
IS6 = float(2.0 ** -6)       # 1/64
S6 = 64.0

F32 = mybir.dt.float32
F32R = mybir.dt.float32r
F16 = mybir.dt.float16
FP8 = mybir.dt.float8e4
AF = mybir.ActivationFunctionType
ALU = mybir.AluOpType
DR = mybir.MatmulPerfMode.DoubleRow

_CACHE = {}


def _f32(ap):
    return ap.bitcast(F32)


def _build_program(rep=1, trace_sim=False, skip_cc=False, upto='all'):
    nc = bacc.Bacc("TRN2", target_bir_lowering=False, debug=False,
                   num_devices=N_CORES)

    # ---- I/O ---------------------------------------------------------------
    hid_shard = nc.dram_tensor("hid_shard", [SEQ, CW], F32,
                               kind="ExternalInput")
    # qk weights: [m_tile, slab, 128, 2, 128] fp8 pairs (column-permuted)
    w_qk8 = nc.dram_tensor("w_qk8", [8, 128, 2 * NS, 128], FP8,
                           kind="ExternalInput")
    w_qkr8 = nc.dram_tensor("w_qkr8", [8, 128, 2 * NS, 128], FP8,
                            kind="ExternalInput")
    w_v8 = nc.dram_tensor("w_v8", [NS // 4, 128, 8, CW], FP8,
                          kind="ExternalInput")
    w_vr8 = nc.dram_tensor("w_vr8", [NS // 4, 128, 8, CW], FP8,
                           kind="ExternalInput")
    w_d8 = nc.dram_tensor("w_d8", [NS, 128, 2, CW], FP8, kind="ExternalInput")
    w_dr8 = nc.dram_tensor("w_dr8", [NS, 128, 2, CW], FP8,
                           kind="ExternalInput")
    b_qk = nc.dram_tensor("b_qk", [128, 8], F32, kind="ExternalInput")
    b_v = nc.dram_tensor("b_v", [1, CW], F16, kind="ExternalInput")   # x64
    b_d = nc.dram_tensor("b_d", [1, CW], F16, kind="ExternalInput")   # x64
    cos_in = nc.dram_tensor("cos_in", [128, SEQ], F16, kind="ExternalInput")
    sin_in = nc.dram_tensor("sin_in", [128, SEQ], F16, kind="ExternalInput")
    mask_in = nc.dram_tensor("mask_in", [128, 4 * SB], mybir.dt.bfloat16,
                             kind="ExternalInput")
    ident_in = nc.dram_tensor("ident_in", [128, 128], F32,
                              kind="ExternalInput")
    ones_col_in = nc.dram_tensor("ones_col_in", [128, 1], F32R,
                                 kind="ExternalInput")
    ones_row_in = nc.dram_tensor("ones_row_in", [1, 128], F32R,
                                 kind="ExternalInput")
    ones_row16_in = nc.dram_tensor("ones_row16_in", [1, 128], F16,
                                   kind="ExternalInput")
    out = nc.dram_tensor("out", [SEQ, CW], F32, kind="ExternalOutput")

    rg = [list(range(N_CORES))]

    with tile.TileContext(nc, trace_sim=trace_sim) as tc:
        with (
            tc.tile_pool(name="const", bufs=1) as constp,
            tc.tile_pool(name="dram", bufs=1, space="DRAM") as dramp,
        ):
            # constants
            ident = constp.tile([128, 128], F32)
            ones_col = constp.tile([128, 1], F32R)
            ones_row = constp.tile([1, 128], F32R)
            ones_row16 = constp.tile([1, 128], F16)
            bqk_sb = constp.tile([128, 8], F32)
            bv_sb = constp.tile([1, CW], F16)
            bd_sb = constp.tile([1, CW], F16)
            cos_sb = constp.tile([128, SEQ], F16)
            sin_sb = constp.tile([128, SEQ], F16)
            nc.sync.dma_start(ident[:], ident_in[:])
            nc.sync.dma_start(ones_col[:], ones_col_in[:])
            nc.sync.dma_start(ones_row[:], ones_row_in[:])
            nc.sync.dma_start(ones_row16[:], ones_row16_in[:])
            nc.sync.dma_start(bqk_sb[:], b_qk[:])
            nc.sync.dma_start(bv_sb[:], b_v[:])
            nc.sync.dma_start(bd_sb[:], b_d[:])
            nc.sync.dma_start(cos_sb[:], cos_in[:])
            nc.sync.dma_start(sin_sb[:], sin_in[:])

            for _rep in range(rep):
              # collective bounce buffers, one per sequence block so each
              # AllGather chunk can overlap compute (fresh per rep).
              # rows [0,CW) = x8 / ctx8, rows [CW,2CW) = 64*residual fp8.
              ccin_h = [dramp.tile([2 * CW, SB], FP8, name=f"ccin_h{_rep}_{i}")
                        for i in range(NSB)]
              ccout_h = [dramp.tile([2 * HIDDEN, SB], FP8, addr_space="Shared",
                                    name=f"ccout_h{_rep}_{i}")
                         for i in range(NSB)]
              ccin_ctx = [dramp.tile([2 * CW, SB], FP8,
                                     name=f"ccin_ctx{_rep}_{i}")
                          for i in range(NSB)]
              ccout_ctx = [dramp.tile([2 * HIDDEN, SB], FP8,
                                      addr_space="Shared",
                                      name=f"ccout_ctx{_rep}_{i}")
                           for i in range(NSB)]

              # ---- P0: transpose own shard, quantize to fp8 pair, AG -------
              with (
                  tc.tile_pool(name="p0sb", bufs=4) as p0sb,
                  tc.tile_pool(name="p0r", bufs=8) as p0r,
                  tc.tile_pool(name="p0ps", bufs=2, space="PSUM") as p0ps,
              ):
                  for sb in range(NSB):
                      for st4 in range(4):
                          st = sb * 4 + st4
                          hs_t = p0sb.tile([128, CW], F32, name="hs_t")
                          nc.sync.dma_start(
                              hs_t[:], hid_shard[st * 128:(st + 1) * 128, :])
                          x8_t = p0sb.tile([128, CW], FP8, name="x8_t")
                          xr8_t = p0sb.tile([128, CW], FP8, name="xr8_t")
                          for kb in range(CW // 128):
                              tp = p0ps.tile([128, 128], F32, name="tp")
                              nc.tensor.transpose(
                                  tp[:], hs_t[:, kb * 128:(kb + 1) * 128],
                                  ident[:])
                              nc.scalar.activation(
                                  x8_t[:, kb * 128:(kb + 1) * 128], tp[:],
                                  AF.Copy)
                              rt = p0r.tile([128, 128], F16, name="rt")
                              nc.vector.tensor_sub(
                                  rt[:], tp[:],
                                  x8_t[:, kb * 128:(kb + 1) * 128])
                              nc.scalar.activation(
                                  xr8_t[:, kb * 128:(kb + 1) * 128], rt[:],
                                  AF.Copy, scale=S6)
                          # batched writes: [128,(kb c)] -> rows kb*128+c
                          nc.gpsimd.dma_start(
                              ccin_h[sb][0:CW,
                                         st4 * 128:(st4 + 1) * 128].rearrange(
                                  "(kb c) s -> c kb s", kb=4),
                              x8_t[:].rearrange("c (kb s) -> c kb s", kb=4))
                          nc.gpsimd.dma_start(
                              ccin_h[sb][CW:2 * CW,
                                         st4 * 128:(st4 + 1) * 128].rearrange(
                                  "(kb c) s -> c kb s", kb=4),
                              xr8_t[:].rearrange("c (kb s) -> c kb s", kb=4))
                      if not skip_cc:
                          nc.gpsimd.collective_compute(
                              "AllGather", mybir.AluOpType.bypass,
                              replica_groups=rg,
                              ins=[ccin_h[sb][:].opt()],
                              outs=[ccout_h[sb][:].opt()])

              # persistent QKV outputs (live through P1+P2)
              with tc.tile_pool(name="qkvout", bufs=1) as qkvp:
                  qh = [qkvp.tile([128, SEQ], F16, name=f"qh{h}")
                        for h in range(HPC)]
                  kh = [qkvp.tile([128, SEQ], F16, name=f"kh{h}")
                        for h in range(HPC)]
                  vsb = [qkvp.tile([128, CW], F16, name=f"v{s}")
                         for s in range(NST)]

                  # ---- P1: QKV projection (fp8 DR, 3-pass) -----------------
                  with (
                      tc.tile_pool(name="htp", bufs=16) as htp,
                      tc.tile_pool(name="wqp", bufs=4) as wqp,
                      tc.tile_pool(name="wvp", bufs=3) as wvp,
                      tc.tile_pool(name="rotp", bufs=2) as rotp,
                      tc.tile_pool(name="rscp", bufs=4) as rscp,
                      tc.tile_pool(name="cmb", bufs=3) as cmbp,
                      tc.tile_pool(name="qkps", bufs=2, space="PSUM") as qkps,
                      tc.tile_pool(name="vps", bufs=2, space="PSUM") as vps,
                  ):
                      def rope(rot_t, dst, sb):
                          """rot_t: [128, SB], rows hl*32+d = rotary dim d of
                          head hl. rotate_half via a partition-permuting
                          SBUF->SBUF DMA; sign lives in the sin table."""
                          cs = cos_sb[:, sb * SB:(sb + 1) * SB]
                          sn = sin_sb[:, sb * SB:(sb + 1) * SB]
                          shf = rscp.tile([128, SB], F32R, name="rsc")
                          for hl in range(HPC):
                              r = hl * ROT
                              nc.gpsimd.dma_start(shf[r:r + HALF, :],
                                                  rot_t[r + HALF:r + ROT, :])
                              nc.gpsimd.dma_start(shf[r + HALF:r + ROT, :],
                                                  rot_t[r:r + HALF, :])
                          t1 = rscp.tile([128, SB], F32R, name="rsc")
                          t2 = rscp.tile([128, SB], F32R, name="rsc")
                          rp = rscp.tile([128, SB], F32R, name="rsc")
                          nc.vector.tensor_mul(t1[:], _f32(rot_t[:]), cs)
                          nc.vector.tensor_mul(t2[:], _f32(shf[:]), sn)
                          nc.vector.tensor_add(rp[:], _f32(t1[:]), _f32(t2[:]))
                          for hl in range(HPC):
                              nc.scalar.activation(
                                  dst[hl][0:ROT, sb * SB:(sb + 1) * SB],
                                  rp[hl * ROT:(hl + 1) * ROT, :], AF.Copy)

                      def evac_qk(m, comb, sb):
                          """comb: [128, SB] f32r = 64*(Wx+...); scale 1/64 and
                          add bias while evacuating."""
                          scols = slice(sb * SB, (sb + 1) * SB)
                          if m == 0 or m == 1:
                              rot_t = rotp.tile([128, SB], F32R, name="rot_t")
                              nc.scalar.activation(rot_t[:], comb[:],
                                                   AF.Identity,
                                                   bias=bqk_sb[:, m:m + 1],
                                                   scale=IS6)
                              rope(rot_t, qh if m == 0 else kh, sb)
                          else:
                              # 32-row chunks: compute-engine partition
                              # accesses >32 rows must start at partition 0;
                              # head spans (96 rows) are exactly 3 chunks.
                              t = (m - 2) % 3
                              dst = qh if m <= 4 else kh
                              for ch in range(4):
                                  g = t * 128 + ch * 32
                                  hl = g // 96
                                  dlo = 32 + g - hl * 96
                                  nc.scalar.activation(
                                      dst[hl][dlo:dlo + 32, scols],
                                      comb[ch * 32:(ch + 1) * 32, :],
                                      AF.Identity,
                                      bias=bqk_sb[ch * 32:(ch + 1) * 32,
                                                  m:m + 1],
                                      scale=IS6)

                      for sb in range(NSB):
                          # hidden^T fp8 pair for this s-block: 8+8 tiles of
                          # [128, 4, SB]
                          hx8, hxr8 = [], []
                          for kg in range(8):
                              t8 = htp.tile([128, 4, SB], FP8, name="hx8")
                              nc.sync.dma_start(
                                  t8[:],
                                  ccout_h[sb][kg * 1024:kg * 1024 + 512,
                                              :].rearrange(
                                      "(k p) s -> p k s", k=4))
                              hx8.append(t8)
                              tr8 = htp.tile([128, 4, SB], FP8, name="hxr8")
                              nc.sync.dma_start(
                                  tr8[:],
                                  ccout_h[sb][kg * 1024 + 512:
                                              (kg + 1) * 1024, :].rearrange(
                                      "(k p) s -> p k s", k=4))
                              hxr8.append(tr8)

                          def xmov(s, which):
                              # moving [128, 2, SB] for slab s
                              g = hx8 if which == 0 else hxr8
                              i = (2 * s) % 4
                              return g[s // 2][:, i:i + 2, :]

                          def xst(s, q4, which):
                              # stationary [128, 2, 128] for slab s, seq q4
                              g = hx8 if which == 0 else hxr8
                              i = (2 * s) % 4
                              return g[s // 2][:, i:i + 2,
                                               q4 * 128:(q4 + 1) * 128]

                          def v_part():
                              # 2 q4-halves: 2 lo + 2 hi psum banks each;
                              # weights streamed per half
                              for hf in range(2):
                                  q4s = (2 * hf, 2 * hf + 1)
                                  plo = {}
                                  phi = {}
                                  for q4 in q4s:
                                      plo[q4] = vps.tile([128, CW], F32,
                                                         name="pvlo")
                                      phi[q4] = vps.tile([128, CW], F32,
                                                         name="pvhi")
                                  for s in range(NS):
                                      if s % 4 == 0:
                                          wv8b = wvp.tile([128, 8, CW], FP8,
                                                          name="wv8b")
                                          nc.sync.dma_start(
                                              wv8b[:], w_v8[s // 4].opt())
                                          wvr8b = wvp.tile([128, 8, CW], FP8,
                                                           name="wvr8b")
                                          nc.sync.dma_start(
                                              wvr8b[:], w_vr8[s // 4].opt())
                                      u = s % 4
                                      wv = wv8b[:, 2 * u:2 * u + 2, :]
                                      wvr = wvr8b[:, 2 * u:2 * u + 2, :]
                                      for q4 in q4s:
                                          nc.tensor.matmul(
                                              plo[q4][:], xst(s, q4, 1), wv,
                                              start=(s == 0), stop=False,
                                              perf_mode=DR)
                                          nc.tensor.matmul(
                                              phi[q4][:], xst(s, q4, 0), wv,
                                              start=(s == 0), stop=False,
                                              perf_mode=DR)
                                          nc.tensor.matmul(
                                              plo[q4][:], xst(s, q4, 0), wvr,
                                              start=False, stop=(s == NS - 1),
                                              perf_mode=DR)
                                  for q4 in q4s:
                                      nc.tensor.matmul(
                                          phi[q4][:], ones_row16[:], bv_sb[:],
                                          start=False, stop=True)
                                      lotmp = cmbp.tile([128, CW], F16,
                                                        name="lotmp")
                                      nc.scalar.activation(
                                          lotmp[:], plo[q4][:], AF.Copy,
                                          scale=IS6)
                                      comb = cmbp.tile([128, CW], F32R,
                                                       name="cmb")
                                      nc.vector.tensor_add(
                                          comb[:], phi[q4][:], lotmp[:])
                                      nc.scalar.activation(
                                          vsb[sb * 4 + q4][:], comb[:],
                                          AF.Copy, scale=IS6)

                          def qk_part():
                              for m in range(8):
                                  wq8 = wqp.tile([128, 2 * NS, 128], FP8,
                                                 name="wq8")
                                  nc.sync.dma_start(wq8[:], w_qk8[m].opt())
                                  wqr8 = wqp.tile([128, 2 * NS, 128], FP8,
                                                  name="wqr8")
                                  nc.sync.dma_start(wqr8[:], w_qkr8[m].opt())
                                  plo = qkps.tile([128, SB], F32, name="pqlo")
                                  phi = qkps.tile([128, SB], F32, name="pqhi")
                                  for s in range(NS):
                                      w8s = wq8[:, 2 * s:2 * s + 2, :]
                                      wr8s = wqr8[:, 2 * s:2 * s + 2, :]
                                      nc.tensor.matmul(
                                          plo[:], w8s, xmov(s, 1),
                                          start=(s == 0), stop=False,
                                          perf_mode=DR)
                                      nc.tensor.matmul(
                                          phi[:], w8s, xmov(s, 0),
                                          start=(s == 0), stop=(s == NS - 1),
                                          perf_mode=DR)
                                      nc.tensor.matmul(
                                          plo[:], wr8s, xmov(s, 0),
                                          start=False, stop=(s == NS - 1),
                                          perf_mode=DR)
                                  lotmp = cmbp.tile([128, SB], F16,
                                                    name="lotmp")
                                  nc.scalar.activation(lotmp[:], plo[:],
                                                       AF.Copy, scale=IS6)
                                  comb = cmbp.tile([128, SB], F32R,
                                                   name="cmb")
                                  nc.vector.tensor_add(comb[:], phi[:],
                                                       lotmp[:])
                                  evac_qk(m, comb, sb)

                          if upto in ('v', 'qk', 'p2', 'all'):
                              v_part()
                          if upto in ('qk', 'p2', 'all'):
                              qk_part()

                  # ---- W_dense prefetch + P2 + P4 (wdp pool spans both) ----
                  wdp_ctx = tc.tile_pool(name="wdp", bufs=1)
                  wdp = wdp_ctx.__enter__()
                  wd_sb = []
                  wdr_sb = []
                  for s in range(NS):
                      w_t = wdp.tile([128, 2, CW], FP8, name=f"wd{s}")
                      nc.sync.dma_start(w_t[:], w_d8[s].opt())
                      wd_sb.append(w_t)
                      wr_t = wdp.tile([128, 2, CW], FP8, name=f"wdr{s}")
                      nc.sync.dma_start(wr_t[:], w_dr8[s].opt())
                      wdr_sb.append(wr_t)

                  # ---- P2: attention ---------------------------------------
                  if upto in ('p2', 'all'):
                   with (
                      tc.tile_pool(name="maskp", bufs=1) as maskp,
                      tc.tile_pool(name="exp", bufs=4) as exp_p,
                      tc.tile_pool(name="accp", bufs=3) as accp,
                      tc.tile_pool(name="rcp", bufs=3) as rcp,
                      tc.tile_pool(name="rbp", bufs=3) as rbp,
                      tc.tile_pool(name="ctxp", bufs=4) as ctxp,
                      tc.tile_pool(name="sps", bufs=2, space="PSUM") as sps,
                      tc.tile_pool(name="cps", bufs=2, space="PSUM") as cps,
                      tc.tile_pool(name="dps", bufs=1, space="PSUM") as dps,
                      tc.tile_pool(name="rbps", bufs=1, space="PSUM") as rbps,
                  ):
                      mask_sb = maskp.tile([128, 4 * SB], mybir.dt.bfloat16)
                      nc.sync.dma_start(mask_sb[:], mask_in[:])

                      for ib in range(NSB):
                          for h in range(HPC):
                              icols = slice(ib * SB, (ib + 1) * SB)
                              njt = 4 * (ib + 1)
                              cp = cps.tile([128, SB], F32, name="cp")
                              acc = accp.tile([128, SB], F32R, name="acc")
                              for jp in range(njt // 2):
                                  # two j-tiles share one [128, 2*SB] psum so
                                  # exp and the denominator add run once per
                                  # pair (ACT is the P2 bottleneck)
                                  sp = sps.tile([128, 2 * SB], F32, name="sp")
                                  for u in range(2):
                                      jt = 2 * jp + u
                                      nc.tensor.matmul(
                                          sp[:, u * SB:(u + 1) * SB],
                                          kh[h][:, jt * 128:(jt + 1) * 128],
                                          qh[h][:, icols], start=True,
                                          stop=True)
                                  if 2 * jp + 1 >= 4 * ib:
                                      t = 2 * jp - 4 * ib
                                      nc.vector.tensor_add(
                                          sp[:], sp[:],
                                          mask_sb[:, t * SB:(t + 2) * SB])
                                  ex = exp_p.tile([128, 2 * SB], F16,
                                                  name="ex")
                                  nc.scalar.activation(ex[:], sp[:], AF.Exp,
                                                       scale=SCALE)
                                  if jp == 0:
                                      nc.vector.tensor_add(
                                          acc[:], ex[:, 0:SB],
                                          ex[:, SB:2 * SB])
                                  else:
                                      nc.vector.tensor_add(
                                          acc[:], _f32(acc[:]),
                                          ex[:, 0:SB])
                                      nc.vector.tensor_add(
                                          acc[:], _f32(acc[:]),
                                          ex[:, SB:2 * SB])
                                  for u in range(2):
                                      jt = 2 * jp + u
                                      nc.tensor.matmul(
                                          cp[:],
                                          vsb[jt][:, h * 128:(h + 1) * 128],
                                          ex[:, u * SB:(u + 1) * SB],
                                          start=(jt == 0),
                                          stop=(jt == njt - 1))
                              dn = dps.tile([1, SB], F32, name="dn")
                              nc.tensor.matmul(dn[:], ones_col[:], acc[:],
                                               start=True, stop=True)
                              rc = rcp.tile([1, SB], F32R, name="rc")
                              with nc.allow_low_precision(
                                      reason="f32r: 11-bit mantissa is plenty "
                                             "for the softmax denominator"):
                                  nc.vector.reciprocal(rc[:], dn[:])
                              rb = rbps.tile([128, SB], F32, name="rb")
                              nc.tensor.matmul(rb[:], ones_row[:], rc[:],
                                               start=True, stop=True)
                              rbs = rbp.tile([128, SB], F32R, name="rbs")
                              nc.scalar.activation(rbs[:], rb[:], AF.Copy)
                              ctxf = ctxp.tile([128, SB], F32R, name="ctxf")
                              nc.vector.tensor_mul(ctxf[:], cp[:],
                                                   _f32(rbs[:]))
                              ctx8 = ctxp.tile([128, SB], FP8, name="ctx8")
                              nc.scalar.activation(ctx8[:], ctxf[:], AF.Copy)
                              rres = ctxp.tile([128, SB], F16, name="rres")
                              nc.vector.tensor_sub(rres[:], _f32(ctxf[:]),
                                                   ctx8[:])
                              ctxr8 = ctxp.tile([128, SB], FP8, name="ctxr8")
                              nc.scalar.activation(ctxr8[:], rres[:],
                                                   AF.Copy, scale=S6)
                              nc.gpsimd.dma_start(
                                  ccin_ctx[ib][h * 128:(h + 1) * 128, :],
                                  ctx8[:])
                              nc.gpsimd.dma_start(
                                  ccin_ctx[ib][CW + h * 128:
                                               CW + (h + 1) * 128, :],
                                  ctxr8[:])
                          if not skip_cc:
                              nc.gpsimd.collective_compute(
                                  "AllGather", mybir.AluOpType.bypass,
                                  replica_groups=rg,
                                  ins=[ccin_ctx[ib][:].opt()],
                                  outs=[ccout_ctx[ib][:].opt()])

                  # ---- P4: dense projection (column shard, fp8 DR) ---------
                  if upto == 'all':
                   with (
                      tc.tile_pool(name="ctp", bufs=18) as ctp,
                      tc.tile_pool(name="outp", bufs=3) as outp,
                      tc.tile_pool(name="cmb2", bufs=3) as cmb2p,
                      tc.tile_pool(name="pdps", bufs=2, space="PSUM") as pdps,
                  ):
                      for mq in range(NSB):
                          c8g, cr8g = [], []
                          for kg in range(8):
                              t8 = ctp.tile([128, 4, SB], FP8, name="c8g")
                              nc.sync.dma_start(
                                  t8[:],
                                  ccout_ctx[mq][kg * 1024:kg * 1024 + 512,
                                                :].rearrange(
                                      "(k p) s -> p k s", k=4))
                              c8g.append(t8)
                              tr8 = ctp.tile([128, 4, SB], FP8, name="cr8g")
                              nc.sync.dma_start(
                                  tr8[:],
                                  ccout_ctx[mq][kg * 1024 + 512:
                                                (kg + 1) * 1024, :].rearrange(
                                      "(k p) s -> p k s", k=4))
                              cr8g.append(tr8)

                          def cst(s, m4, which):
                              g = c8g if which == 0 else cr8g
                              i = (2 * s) % 4
                              return g[s // 2][:, i:i + 2,
                                               m4 * 128:(m4 + 1) * 128]

                          for hf in range(2):
                              m4s = (2 * hf, 2 * hf + 1)
                              plo = {}
                              phi = {}
                              for m4 in m4s:
                                  plo[m4] = pdps.tile([128, CW], F32,
                                                      name="pdlo")
                                  phi[m4] = pdps.tile([128, CW], F32,
                                                      name="pdhi")
                              for s in range(NS):
                                  for m4 in m4s:
                                      nc.tensor.matmul(
                                          plo[m4][:], cst(s, m4, 1),
                                          wd_sb[s][:], start=(s == 0),
                                          stop=False, perf_mode=DR)
                                      nc.tensor.matmul(
                                          phi[m4][:], cst(s, m4, 0),
                                          wd_sb[s][:], start=(s == 0),
                                          stop=False, perf_mode=DR)
                                      nc.tensor.matmul(
                                          plo[m4][:], cst(s, m4, 0),
                                          wdr_sb[s][:], start=False,
                                          stop=(s == NS - 1), perf_mode=DR)
                              for m4 in m4s:
                                  nc.tensor.matmul(
                                      phi[m4][:], ones_row16[:], bd_sb[:],
                                      start=False, stop=True)
                                  lotmp = cmb2p.tile([128, CW], F16,
                                                     name="lotmp2")
                                  nc.scalar.activation(lotmp[:], plo[m4][:],
                                                       AF.Copy, scale=IS6)
                                  comb = cmb2p.tile([128, CW], F32R,
                                                    name="cmb2")
                                  nc.vector.tensor_add(comb[:], phi[m4][:],
                                                       lotmp[:])
                                  ot = outp.tile([128, CW], F32, name="ot")
                                  nc.scalar.activation(ot[:], comb[:],
                                                       AF.Copy, scale=IS6)
                                  st = mq * 4 + m4
                                  nc.sync.dma_start(
                                      out[st * 128:(st + 1) * 128, :], ot[:])

                  wdp_ctx.__exit__(None, None, None)

    nc.compile()
    return nc


def _get_exec(rep=1):
    if ("exec", rep) in _CACHE:
        return _CACHE[("exec", rep)]
    import jax
    from jax.sharding import Mesh, PartitionSpec
    from jax.experimental.shard_map import shard_map
    from concourse import bass2jax

    nc = _build_program(rep=rep)
    bass2jax.install_neuronx_cc_hook()

    partition_name = (nc.partition_id_tensor.name
                      if nc.partition_id_tensor else None)
    in_names = []
    out_names = []
    out_avals = []
    zero_shapes = []
    for alloc in nc.m.functions[0].allocations:
        if not isinstance(alloc, mybir.MemoryLocationSet):
            continue
        name = alloc.memorylocations[0].name
        if alloc.kind == "ExternalInput":
            if name != partition_name:
                in_names.append(name)
        elif alloc.kind == "ExternalOutput":
            np_dt = mybir.dt.np(alloc.dtype)
            out_names.append(name)
            out_avals.append(
                jax.core.ShapedArray(tuple(alloc.tensor_shape), np_dt))
            zero_shapes.append((tuple(alloc.tensor_shape), np_dt))

    n_params = len(in_names)
    n_outs = len(out_names)
    all_in_names = in_names + out_names
    if partition_name is not None:
        all_in_names = all_in_names + [partition_name]
    donate = tuple(range(n_params, n_params + n_outs))

    def _body(*args):
        operands = list(args)
        if partition_name is not None:
            operands.append(bass2jax.partition_id_tensor())
        outs = bass2jax._bass_exec_p.bind(
            *operands,
            out_avals=tuple(out_avals),
            in_names=tuple(all_in_names),
            out_names=tuple(out_names),
            lowering_input_output_aliases=(),
            sim_require_finite=True,
            sim_require_nnan=True,
            nc=nc,
        )
        return tuple(outs)

    devices = jax.devices()[:N_CORES]
    mesh = Mesh(np.asarray(devices), ("core",))
    in_specs = (PartitionSpec("core"),) * (n_params + n_outs)
    out_specs = (PartitionSpec("core"),) * n_outs
    sharded = jax.jit(
        shard_map(_body, mesh=mesh, in_specs=in_specs, out_specs=out_specs,
                  check_rep=False),
        donate_argnums=donate, keep_unused=True)

    _CACHE[("nc", rep)] = nc
    _CACHE[("exec", rep)] = (sharded, in_names, out_names, out_avals,
                             zero_shapes)
    return _CACHE[("exec", rep)]


def _run_cores(in_maps):
    """Run the SPMD program; in_maps is a list of 8 dicts name->np.ndarray."""
    sharded, in_names, out_names, out_avals, zero_shapes = _get_exec()
    concat_in = [
        np.concatenate([np.asarray(in_maps[c][n]) for c in range(N_CORES)],
                       axis=0)
        for n in in_names
    ]
    concat_zeros = [
        np.zeros((N_CORES * s[0], *s[1:]), dt) for (s, dt) in zero_shapes
    ]
    out_arrs = sharded(*concat_in, *concat_zeros)
    return [
        {n: np.asarray(out_arrs[i]).reshape(N_CORES, *out_avals[i].shape)[c]
         for i, n in enumerate(out_names)}
        for c in range(N_CORES)
    ]


def benchmark(in_maps, iters=10, rep=1):
    """Time repeated executions with device-resident inputs. Returns list of
    per-call wall seconds (axon RPC overhead included)."""
    import time
    import jax
    import jax.numpy as jnp
    from jax.sharding import Mesh, PartitionSpec, NamedSharding

    sharded, in_names, out_names, out_avals, zero_shapes = _get_exec(rep)
    devices = jax.devices()[:N_CORES]
    mesh = Mesh(np.asarray(devices), ("core",))
    shard = NamedSharding(mesh, PartitionSpec("core"))
    dev_in = [
        jax.device_put(
            np.concatenate([np.asarray(in_maps[c][n]) for c in range(N_CORES)],
                           axis=0), shard)
        for n in in_names
    ]
    jax.block_until_ready(dev_in)

    def make_zeros():
        zs = [jnp.zeros((N_CORES * s[0], *s[1:]), dt, device=shard)
              for (s, dt) in zero_shapes]
        jax.block_until_ready(zs)
        return zs

    out = sharded(*dev_in, *make_zeros())
    jax.block_until_ready(out)
    times = []
    for _ in range(iters):
        zs = make_zeros()
        t0 = time.perf_counter()
        out = sharded(*dev_in, *zs)
        jax.block_until_ready(out)
        times.append(time.perf_counter() - t0)
    return times


def _qpair(W):
    """W: [4096, C] f32 -> (W8, Wr8) fp8e4 arrays, scale-64 pair layout
    [NS, 128, 2, C]."""
    import ml_dtypes
    E4 = ml_dtypes.float8_e4m3
    C = W.shape[1]
    Ws = (W * S6).astype(np.float32)
    W8 = Ws.astype(E4)
    Wr8 = ((Ws - W8.astype(np.float32)) * S6).astype(E4)

    def pack(A):
        return np.ascontiguousarray(
            A.reshape(NS, 2, 128, C).transpose(0, 2, 1, 3))

    def pack4(A):
        # group 4 slabs: [NS//4, 128, (4 slab x 2 pair), C]
        return np.ascontiguousarray(
            A.reshape(NS // 4, 4, 2, 128, C).transpose(0, 3, 1, 2, 4).reshape(
                NS // 4, 128, 8, C))

    return pack(W8), pack(Wr8), pack4(W8), pack4(Wr8)


def _qpair_qk(W):
    """W: [4096, 1024] (permuted q|k cols) -> [8, NS, 128, 2, 128] pair."""
    import ml_dtypes
    E4 = ml_dtypes.float8_e4m3
    Ws = (W * S6).astype(np.float32)
    W8 = Ws.astype(E4)
    Wr8 = ((Ws - W8.astype(np.float32)) * S6).astype(E4)

    def pack(A):
        # [4096, 1024] -> [m 8, kpart 128, (slab 16 x pair 2), 128]
        return np.ascontiguousarray(
            A.reshape(NS, 2, 128, 8, 128).transpose(3, 2, 0, 1, 4).reshape(
                8, 128, 2 * NS, 128))

    return pack(W8), pack(Wr8)


def _host_prep(hidden_states, W_qkv, b_qkv, W_dense, b_dense):
    hid = np.ascontiguousarray(
        np.asarray(hidden_states, dtype=np.float32).reshape(SEQ, HIDDEN))
    W_qkv = np.asarray(W_qkv, dtype=np.float32)
    b_qkv = np.asarray(b_qkv, dtype=np.float32)
    W_dense = np.asarray(W_dense, dtype=np.float32)
    b_dense = np.asarray(b_dense, dtype=np.float32)

    # rotary tables, computed in float32 exactly as the reference does
    inv_freq = (1.0 / (np.float32(10000.0) **
                       (np.arange(0, ROT, 2, dtype=np.float32)
                        / np.float32(ROT))))
    t = np.arange(SEQ, dtype=np.float32)
    freqs = t[:, None] * inv_freq[None, :]          # [SEQ, 16]
    cosf = np.cos(freqs).T                          # [16, SEQ]
    sinf = np.sin(freqs).T
    # row hl*32 + d: cos(emb[d mod 16]); sin carries the rotate_half sign
    cos_blk = np.concatenate([cosf, cosf], axis=0)      # [32, SEQ]
    sin_blk = np.concatenate([-sinf, sinf], axis=0)
    cos_t = np.tile(cos_blk, (HPC, 1)).astype(np.float16)  # [128, SEQ]
    sin_t = np.tile(sin_blk, (HPC, 1)).astype(np.float16)

    # additive causal masks for the 4 diagonal j-tiles of each i-block
    pj = np.arange(128)[:, None]
    fi = np.arange(SB)[None, :]
    mask = np.concatenate(
        [np.where(128 * t_ + pj <= fi, 0.0, NEG) for t_ in range(4)],
        axis=1).astype(__import__('ml_dtypes').bfloat16)

    ident = np.eye(128, dtype=np.float32)

    in_maps = []
    for c in range(N_CORES):
        heads = [HPC * c + i for i in range(HPC)]
        qcol = lambda h, d: h * 3 * HD + d
        kcol = lambda h, d: h * 3 * HD + HD + d
        vcol = lambda h, d: h * 3 * HD + 2 * HD + d
        perm = []
        perm += [qcol(h, d) for h in heads for d in range(ROT)]
        perm += [kcol(h, d) for h in heads for d in range(ROT)]
        perm += [qcol(h, d) for h in heads for d in range(ROT, HD)]
        perm += [kcol(h, d) for h in heads for d in range(ROT, HD)]
        perm = np.asarray(perm)
        vperm = np.asarray([vcol(h, d) for h in heads for d in range(HD)])

        w_qk8, w_qkr8 = _qpair_qk(W_qkv[:, perm])
        _, _, w_v8, w_vr8 = _qpair(W_qkv[:, vperm])
        w_d8, w_dr8, _, _ = _qpair(W_dense[:, c * CW:(c + 1) * CW])
        in_maps.append({
            "hid_shard": np.ascontiguousarray(hid[:, c * CW:(c + 1) * CW]),
            "w_qk8": w_qk8,
            "w_qkr8": w_qkr8,
            "w_v8": w_v8,
            "w_vr8": w_vr8,
            "w_d8": w_d8,
            "w_dr8": w_dr8,
            "b_qk": np.ascontiguousarray(b_qkv[perm].reshape(8, 128).T),
            "b_v": (b_qkv[vperm] * S6).astype(np.float16).reshape(1, CW),
            "b_d": (b_dense[c * CW:(c + 1) * CW] * S6).astype(
                np.float16).reshape(1, CW),
            "cos_in": cos_t,
            "sin_in": sin_t,
            "mask_in": mask,
            "ident_in": ident,
            "ones_col_in": np.ones((128, 1), np.float32),
            "ones_row_in": np.ones((1, 128), np.float32),
            "ones_row16_in": np.ones((1, 128), np.float16),
        })
    return in_maps


def kernel(hidden_states, attention_mask=None, W_qkv=None, b_qkv=None,
           W_dense=None, b_dense=None, **_unused):
    in_maps = _host_prep(hidden_states, W_qkv, b_qkv, W_dense, b_dense)
    results = _run_cores(in_maps)
    full = np.concatenate([results[c]["out"] for c in range(N_CORES)], axis=1)
    return full.reshape(SEQ, 1, HIDDEN).astype(np.float32)


if __name__ == "__main__":
    rng = np.random.default_rng(0)
    ins = {
        "hidden_states": rng.standard_normal((SEQ, 1, HIDDEN),
                                             dtype=np.float32),
        "attention_mask": np.triu(np.ones((SEQ, SEQ), dtype=bool),
                                  1)[None, None],
        "W_qkv": (rng.standard_normal((HIDDEN, 3 * HIDDEN), dtype=np.float32)
                  * 0.02),
        "b_qkv": np.zeros(3 * HIDDEN, np.float32),
        "W_dense": (rng.standard_normal((HIDDEN, HIDDEN), dtype=np.float32)
                    * 0.02),
        "b_dense": np.zeros(HIDDEN, np.float32),
    }
    o = kernel(**ins)
    print("kernel output:", o.shape, o.dtype, float(np.abs(o).max()))
